# revision 17
# baseline (speedup 1.0000x reference)
"""Cluster-loss (two-view) Trainium2 kernel — v2 (sorted segment-sum).

Math:
    f1n = feat1 / ||feat1||_row ;  f2n = feat2 / ||feat2||_row
    hseg[c] = sum_{t: label[t]=c} (f1n - f2n)[t]      (s1 - s2 fused)
    loss = sum_c relu(||hseg[c]/max(count_c,1)||^2 - margin)

Key restructure vs v1: tokens are SORTED BY LABEL on the host (free), so
each 128-token tile spans at most 2 classes (min class count ~910 >> 128).
The [128, 1024] one-hot matmul (the old DVE bottleneck at 95.8% busy)
collapses to a host-built 0/1 mask [128, 2] per tile.  Normalize-and-
subtract is folded INTO the PE accumulation:

    psum[:, 2i:2i+2] = f1_tile^T @ (mask*rs1)  +  f2_tile^T @ (mask*-rs2)

so there is no per-tile elementwise u-build at all.  Each tile owns its
own 2 PSUM columns (976*2 = 1952 <= 4096), no cross-tile accumulation;
host maps slots->classes and finishes (counts, hinge) in numpy.

Per-tile device work (budget = DMA roofline 364 ns @ 358 GB/s):
  - DMA: 128 KB f32 in (host pre-packed to contiguous 8 KB lines),
    SWDGE-cast to f16 in flight.
  - DVE: 2x tensor_tensor_reduce (sum of squares, f16 2x mode) + batched
    w-builds (mask (*) rs broadcast) + reciprocals.
  - ACT: sqrt (batched), final PSUM drain.
  - PE : 2x (LDWEIGHTS f16 + matmul N=2).

Sharding: data-parallel over N; core i gets sorted rows
[i*124928, (i+1)*124928); remainder 576 rows finished on host.
"""

from contextlib import ExitStack

import numpy as np

import concourse.bass as bass
import concourse.mybir as mybir
import concourse.tile as tile
from concourse import bacc
from concourse.bass_utils import run_bass_kernel_spmd

N_CORES = 8
D = 128              # feature dim (matmul M)
C = 1000             # classes
P = 128              # tokens per tile (matmul K)
TPB = 16             # tiles per DMA batch (1 MiB f32 per view per batch)
NB = 61              # batches per core
NT = NB * TPB        # 976 tiles per core
SHARD = NT * P       # 124928 tokens per core
USED = N_CORES * SHARD
NSLOT = 2 * NT       # psum column slots per core (1952 <= 4096)
MARGIN = 0.1

F32 = mybir.dt.float32
F16 = mybir.dt.float16
AF = mybir.ActivationFunctionType
OP = mybir.AluOpType

# sum-of-squares strategy: "ttr" = fused tensor_tensor_reduce per tile on
# DVE; "act" = batched ACT Square + batched DVE tensor_reduce.
SUMSQ = "act"
# feature dtype in SBUF: F32 = plain HWDGE loads; F16 = SWDGE cast-in-flight
# (SWDGE cast hit an INTERNAL error on HW via axon; sim-only validated)
FDT = F16


def build_nc(nb: int = NB):
    nt = nb * TPB
    nslot = 2 * nt
    nc = bacc.Bacc("TRN2", target_bir_lowering=False, debug=False)

    ff_d = nc.dram_tensor("ff", [nb, P, 2 * TPB * D], F32, kind="ExternalInput")
    mask_d = nc.dram_tensor("mask", [P, nt * 2], F16, kind="ExternalInput")
    out_d = nc.dram_tensor("hseg", [D, nslot], F32, kind="ExternalOutput")

    ffr = ff_d.ap()

    with tile.TileContext(nc) as tc, ExitStack() as ctx:
        const = ctx.enter_context(tc.tile_pool(name="const", bufs=1))
        fpool = ctx.enter_context(tc.tile_pool(name="fpool", bufs=8))
        sqpool = ctx.enter_context(tc.tile_pool(name="sqpool", bufs=8))
        spool = ctx.enter_context(tc.tile_pool(name="spool", bufs=5))
        wpool = ctx.enter_context(tc.tile_pool(name="wpool", bufs=5))
        ppool = ctx.enter_context(tc.tile_pool(name="ppool", bufs=1, space="PSUM"))

        mask_sb = const.tile([P, nt, 2], F16)
        nc.sync.dma_start(
            mask_sb[:].rearrange("p i j -> p (i j)"), mask_d.ap()
        )

        psum = ppool.tile([D, 2048], F32)   # 4 banks; cols 0..1951 used
        outsb = const.tile([D, nslot], F32)

        dma_load = nc.gpsimd.dma_start if FDT != F32 else nc.sync.dma_start
        for b in range(nb):
            # one packed DMA per batch: both views, 16 KB contiguous lines
            fft = fpool.tile([P, 2, TPB, D], FDT, name="fft")
            dma_load(fft[:].rearrange("p v t d -> p (v t d)"), ffr[b])
            f1t = fft[:, 0]
            f2t = fft[:, 1]

            # fused sum-of-squares for both views: 1 ACT square + 1 DVE reduce
            sq12 = sqpool.tile([P, 2 * TPB * D], F16, name="sq12")
            nc.scalar.activation(
                sq12[:], fft[:].rearrange("p v t d -> p (v t d)"), AF.Square
            )
            ss12 = spool.tile([P, 2, TPB], F32, name="ss12")
            nc.vector.tensor_reduce(
                ss12[:], sq12[:].rearrange("p (v t d) -> p v t d", t=TPB, d=D),
                axis=mybir.AxisListType.X, op=OP.add,
            )
            rt12 = spool.tile([P, 2, TPB], F32, name="rt12")
            nc.scalar.activation(rt12[:], ss12[:], AF.Sqrt)   # ||f1||,||f2||
            inv12 = spool.tile([P, 2, TPB], F32, name="inv12")
            nc.vector.reciprocal(inv12[:], rt12[:])           # rs1, rs2
            inv1 = inv12[:, 0]
            ninv2 = spool.tile([P, TPB], F32, name="ninv2")
            nc.vector.tensor_scalar(
                out=ninv2[:], in0=inv12[:, 1], scalar1=-1.0, scalar2=None,
                op0=OP.mult,
            )                                                 # -rs2

            mslice = mask_sb[:, b * TPB : (b + 1) * TPB, :]
            w1b = wpool.tile([P, TPB, 2], FDT, name="w1b")
            nc.vector.tensor_tensor(
                w1b[:], mslice,
                inv1[:, :, None].broadcast_to([P, TPB, 2]), OP.mult,
            )
            w2b = wpool.tile([P, TPB, 2], FDT, name="w2b")
            nc.vector.tensor_tensor(
                w2b[:], mslice,
                ninv2[:, :, None].broadcast_to([P, TPB, 2]), OP.mult,
            )

            for t in range(TPB):
                co = 2 * (b * TPB + t)
                nc.tensor.matmul(
                    psum[:, co : co + 2], f1t[:, t, :], w1b[:, t, :],
                    start=True, stop=False,
                )
                nc.tensor.matmul(
                    psum[:, co : co + 2], f2t[:, t, :], w2b[:, t, :],
                    start=False, stop=True,
                )



        for k in range((nslot + 511) // 512):
            lo, hi = k * 512, min((k + 1) * 512, nslot)
            nc.scalar.copy(outsb[:, lo:hi], psum[:, lo:hi])
            nc.sync.dma_start(out_d.ap()[:, lo:hi], outsb[:, lo:hi])

    nc.compile()
    return nc


_NC_CACHE = {}


def _get_nc():
    if "nc" not in _NC_CACHE:
        _NC_CACHE["nc"] = build_nc()
    return _NC_CACHE["nc"]


def make_in_maps(feat1, feat2, label1, order):
    """Sort by label, pack per-core tiles, build per-tile 2-col class masks."""
    lab_s = label1[order]
    in_maps = []
    cls_maps = []
    for c in range(N_CORES):
        sel = order[c * SHARD : (c + 1) * SHARD]
        # pack both views: ff[b, p, v, t, d] so each partition line is one
        # contiguous 16 KB chunk per batch
        ff = np.empty((NB, P, 2, TPB, D), dtype=np.float32)
        for v, feat in enumerate((feat1, feat2)):
            ff[:, :, v] = feat[sel].reshape(NB, TPB, P, D).transpose(0, 2, 1, 3)
        ff = ff.reshape(NB, P, 2 * TPB * D)
        lab = lab_s[c * SHARD : (c + 1) * SHARD].reshape(NT, P)
        first = lab[:, :1]
        second = lab[:, -1:]
        assert ((lab == first) | (lab == second)).all(), (
            "a 128-token tile spans >2 classes; sorted-tile scheme invalid"
        )
        m0 = lab == first
        m1 = (lab == second) & ~m0
        mask = np.stack([m0, m1], axis=-1).astype(np.float16)  # [NT, P, 2]
        mask = np.ascontiguousarray(mask.transpose(1, 0, 2)).reshape(P, NT * 2)
        in_maps.append({"ff": ff, "mask": mask})
        cls_maps.append(
            np.concatenate([first, second], axis=1).reshape(-1)  # [NT*2]
        )
    return in_maps, cls_maps


def finish_host(hseg_list, cls_maps, feat1, feat2, label1, order):
    """Slots -> per-class sums, host remainder, counts, hinge."""
    hseg = np.zeros((C, D), dtype=np.float64)
    for c in range(N_CORES):
        contrib = hseg_list[c].astype(np.float64).T  # [NSLOT, D]
        np.add.at(hseg, cls_maps[c], contrib)
    rem = order[USED:]
    if rem.size:
        r1 = feat1[rem].astype(np.float64)
        r2 = feat2[rem].astype(np.float64)
        n1 = np.sqrt((r1 * r1).sum(1, keepdims=True))
        n2 = np.sqrt((r2 * r2).sum(1, keepdims=True))
        np.add.at(hseg, label1[rem], r1 / n1 - r2 / n2)
    counts = np.bincount(label1, minlength=C).astype(np.float64)
    denom = np.maximum(counts, 1.0)
    cd = hseg / denom[:, None]
    per_class = (cd * cd).sum(1)
    hinge = np.maximum(per_class - MARGIN, 0.0)
    hinge = np.where(counts > 0, hinge, 0.0)
    _DEBUG.update(hseg=hseg, per_class=per_class, counts=counts)
    return np.array(hinge.sum(), dtype=np.float32)


_DEBUG = {}  # dev-only introspection (hseg/per_class of last call)


def kernel(feat1, feat2, label1, trace: bool = False):
    feat1 = np.ascontiguousarray(np.asarray(feat1, dtype=np.float32))
    feat2 = np.ascontiguousarray(np.asarray(feat2, dtype=np.float32))
    label1 = np.asarray(label1).astype(np.int64)

    order = np.argsort(label1, kind="stable")
    in_maps, cls_maps = make_in_maps(feat1, feat2, label1, order)
    nc = _get_nc()
    res = run_bass_kernel_spmd(
        nc, in_maps, core_ids=list(range(N_CORES)), trace=trace
    )
    hsegs = [res.results[i]["hseg"] for i in range(N_CORES)]
    out = finish_host(hsegs, cls_maps, feat1, feat2, label1, order)
    if trace:
        return out, res
    return out


# revision 18
# speedup vs baseline: 1.0139x; 1.0139x over previous
"""Cluster-loss (two-view) Trainium2 kernel — v2 (sorted segment-sum).

Math:
    f1n = feat1 / ||feat1||_row ;  f2n = feat2 / ||feat2||_row
    hseg[c] = sum_{t: label[t]=c} (f1n - f2n)[t]      (s1 - s2 fused)
    loss = sum_c relu(||hseg[c]/max(count_c,1)||^2 - margin)

Key restructure vs v1: tokens are SORTED BY LABEL on the host (free), so
each 128-token tile spans at most 2 classes (min class count ~910 >> 128).
The [128, 1024] one-hot matmul (the old DVE bottleneck at 95.8% busy)
collapses to a host-built 0/1 mask [128, 2] per tile.  Normalize-and-
subtract is folded INTO the PE accumulation:

    psum[:, 2i:2i+2] = f1_tile^T @ (mask*rs1)  +  f2_tile^T @ (mask*-rs2)

so there is no per-tile elementwise u-build at all.  Each tile owns its
own 2 PSUM columns (976*2 = 1952 <= 4096), no cross-tile accumulation;
host maps slots->classes and finishes (counts, hinge) in numpy.

Per-tile device work (budget = DMA roofline 364 ns @ 358 GB/s):
  - DMA: 128 KB f32 in (host pre-packed to contiguous 8 KB lines),
    SWDGE-cast to f16 in flight.
  - DVE: 2x tensor_tensor_reduce (sum of squares, f16 2x mode) + batched
    w-builds (mask (*) rs broadcast) + reciprocals.
  - ACT: sqrt (batched), final PSUM drain.
  - PE : 2x (LDWEIGHTS f16 + matmul N=2).

Sharding: data-parallel over N; core i gets sorted rows
[i*124928, (i+1)*124928); remainder 576 rows finished on host.
"""

from contextlib import ExitStack

import numpy as np

import concourse.bass as bass
import concourse.mybir as mybir
import concourse.tile as tile
from concourse import bacc
from concourse.bass_utils import run_bass_kernel_spmd

N_CORES = 8
D = 128              # feature dim (matmul M)
C = 1000             # classes
P = 128              # tokens per tile (matmul K)
TPB = 16             # tiles per DMA batch (1 MiB f32 per view per batch)
NB = 61              # batches per core
NT = NB * TPB        # 976 tiles per core
SHARD = NT * P       # 124928 tokens per core
USED = N_CORES * SHARD
NSLOT = 2 * NT       # psum column slots per core (1952 <= 4096)
MARGIN = 0.1

F32 = mybir.dt.float32
F16 = mybir.dt.float16
AF = mybir.ActivationFunctionType
OP = mybir.AluOpType

# sum-of-squares strategy: "ttr" = fused tensor_tensor_reduce per tile on
# DVE; "act" = batched ACT Square + batched DVE tensor_reduce.
SUMSQ = "act"
# feature dtype in SBUF: F32 = plain HWDGE loads; F16 = SWDGE cast-in-flight
# (SWDGE cast hit an INTERNAL error on HW via axon; sim-only validated)
FDT = F16


def build_nc(nb: int = NB):
    nt = nb * TPB
    nslot = 2 * nt
    nc = bacc.Bacc("TRN2", target_bir_lowering=False, debug=False)

    ff_d = nc.dram_tensor("ff", [nb, P, 2 * TPB * D], F32, kind="ExternalInput")
    mask_d = nc.dram_tensor("mask", [P, nt * 2], F16, kind="ExternalInput")
    out_d = nc.dram_tensor("hseg", [D, nslot], F32, kind="ExternalOutput")

    ffr = ff_d.ap()

    with tile.TileContext(nc) as tc, ExitStack() as ctx:
        const = ctx.enter_context(tc.tile_pool(name="const", bufs=1))
        fpool = ctx.enter_context(tc.tile_pool(name="fpool", bufs=6))
        sqpool = ctx.enter_context(tc.tile_pool(name="sqpool", bufs=6))
        spool = ctx.enter_context(tc.tile_pool(name="spool", bufs=5))
        wpool = ctx.enter_context(tc.tile_pool(name="wpool", bufs=5))
        ppool = ctx.enter_context(tc.tile_pool(name="ppool", bufs=1, space="PSUM"))

        mask_sb = const.tile([P, nt, 2], F16)
        nc.sync.dma_start(
            mask_sb[:].rearrange("p i j -> p (i j)"), mask_d.ap()
        )

        psum = ppool.tile([D, 2048], F32)   # 4 banks; cols 0..1951 used
        outsb = const.tile([D, nslot], F32)

        dma_load = nc.gpsimd.dma_start if FDT != F32 else nc.sync.dma_start
        for b in range(nb):
            # one packed DMA per batch: both views, 16 KB contiguous lines
            fft = fpool.tile([P, 2, TPB, D], FDT, name="fft")
            dma_load(fft[:].rearrange("p v t d -> p (v t d)"), ffr[b])
            f1t = fft[:, 0]
            f2t = fft[:, 1]

            # fused sum-of-squares for both views: 1 ACT square + 1 DVE reduce
            sq12 = sqpool.tile([P, 2 * TPB * D], F16, name="sq12")
            nc.scalar.activation(
                sq12[:], fft[:].rearrange("p v t d -> p (v t d)"), AF.Square
            )
            ss12 = spool.tile([P, 2, TPB], F32, name="ss12")
            nc.vector.tensor_reduce(
                ss12[:], sq12[:].rearrange("p (v t d) -> p v t d", t=TPB, d=D),
                axis=mybir.AxisListType.X, op=OP.add,
            )
            rt12 = spool.tile([P, 2, TPB], F32, name="rt12")
            nc.scalar.activation(rt12[:], ss12[:], AF.Sqrt)   # ||f1||,||f2||
            inv12 = spool.tile([P, 2, TPB], F32, name="inv12")
            nc.vector.reciprocal(inv12[:], rt12[:])           # rs1, rs2
            inv1 = inv12[:, 0]
            ninv2 = spool.tile([P, TPB], F32, name="ninv2")
            nc.vector.tensor_scalar(
                out=ninv2[:], in0=inv12[:, 1], scalar1=-1.0, scalar2=None,
                op0=OP.mult,
            )                                                 # -rs2

            mslice = mask_sb[:, b * TPB : (b + 1) * TPB, :]
            w1b = wpool.tile([P, TPB, 2], FDT, name="w1b")
            nc.vector.tensor_tensor(
                w1b[:], mslice,
                inv1[:, :, None].broadcast_to([P, TPB, 2]), OP.mult,
            )
            w2b = wpool.tile([P, TPB, 2], FDT, name="w2b")
            nc.vector.tensor_tensor(
                w2b[:], mslice,
                ninv2[:, :, None].broadcast_to([P, TPB, 2]), OP.mult,
            )

            for t in range(TPB):
                co = 2 * (b * TPB + t)
                nc.tensor.matmul(
                    psum[:, co : co + 2], f1t[:, t, :], w1b[:, t, :],
                    start=True, stop=False,
                )
                nc.tensor.matmul(
                    psum[:, co : co + 2], f2t[:, t, :], w2b[:, t, :],
                    start=False, stop=True,
                )

            if b == nb - 2:
                # drain all full banks whose tiles are already complete; the
                # in-stream is winding down so the out-DMA no longer competes
                done_cols = 2 * (b + 1) * TPB
                for k in range((nslot + 511) // 512):
                    lo, hi = k * 512, min((k + 1) * 512, nslot)
                    if hi <= done_cols:
                        nc.scalar.copy(outsb[:, lo:hi], psum[:, lo:hi])
                        nc.sync.dma_start(out_d.ap()[:, lo:hi], outsb[:, lo:hi])



        done_at_penultimate = 2 * (nb - 1) * TPB
        for k in range((nslot + 511) // 512):
            lo, hi = k * 512, min((k + 1) * 512, nslot)
            if hi > done_at_penultimate:
                nc.scalar.copy(outsb[:, lo:hi], psum[:, lo:hi])
                nc.sync.dma_start(out_d.ap()[:, lo:hi], outsb[:, lo:hi])

    nc.compile()
    return nc


_NC_CACHE = {}


def _get_nc():
    if "nc" not in _NC_CACHE:
        _NC_CACHE["nc"] = build_nc()
    return _NC_CACHE["nc"]


def make_in_maps(feat1, feat2, label1, order):
    """Sort by label, pack per-core tiles, build per-tile 2-col class masks."""
    lab_s = label1[order]
    in_maps = []
    cls_maps = []
    for c in range(N_CORES):
        sel = order[c * SHARD : (c + 1) * SHARD]
        # pack both views: ff[b, p, v, t, d] so each partition line is one
        # contiguous 16 KB chunk per batch
        ff = np.empty((NB, P, 2, TPB, D), dtype=np.float32)
        for v, feat in enumerate((feat1, feat2)):
            ff[:, :, v] = feat[sel].reshape(NB, TPB, P, D).transpose(0, 2, 1, 3)
        ff = ff.reshape(NB, P, 2 * TPB * D)
        lab = lab_s[c * SHARD : (c + 1) * SHARD].reshape(NT, P)
        first = lab[:, :1]
        second = lab[:, -1:]
        assert ((lab == first) | (lab == second)).all(), (
            "a 128-token tile spans >2 classes; sorted-tile scheme invalid"
        )
        m0 = lab == first
        m1 = (lab == second) & ~m0
        mask = np.stack([m0, m1], axis=-1).astype(np.float16)  # [NT, P, 2]
        mask = np.ascontiguousarray(mask.transpose(1, 0, 2)).reshape(P, NT * 2)
        in_maps.append({"ff": ff, "mask": mask})
        cls_maps.append(
            np.concatenate([first, second], axis=1).reshape(-1)  # [NT*2]
        )
    return in_maps, cls_maps


def finish_host(hseg_list, cls_maps, feat1, feat2, label1, order):
    """Slots -> per-class sums, host remainder, counts, hinge."""
    hseg = np.zeros((C, D), dtype=np.float64)
    for c in range(N_CORES):
        contrib = hseg_list[c].astype(np.float64).T  # [NSLOT, D]
        np.add.at(hseg, cls_maps[c], contrib)
    rem = order[USED:]
    if rem.size:
        r1 = feat1[rem].astype(np.float64)
        r2 = feat2[rem].astype(np.float64)
        n1 = np.sqrt((r1 * r1).sum(1, keepdims=True))
        n2 = np.sqrt((r2 * r2).sum(1, keepdims=True))
        np.add.at(hseg, label1[rem], r1 / n1 - r2 / n2)
    counts = np.bincount(label1, minlength=C).astype(np.float64)
    denom = np.maximum(counts, 1.0)
    cd = hseg / denom[:, None]
    per_class = (cd * cd).sum(1)
    hinge = np.maximum(per_class - MARGIN, 0.0)
    hinge = np.where(counts > 0, hinge, 0.0)
    _DEBUG.update(hseg=hseg, per_class=per_class, counts=counts)
    return np.array(hinge.sum(), dtype=np.float32)


_DEBUG = {}  # dev-only introspection (hseg/per_class of last call)


def kernel(feat1, feat2, label1, trace: bool = False):
    feat1 = np.ascontiguousarray(np.asarray(feat1, dtype=np.float32))
    feat2 = np.ascontiguousarray(np.asarray(feat2, dtype=np.float32))
    label1 = np.asarray(label1).astype(np.int64)

    order = np.argsort(label1, kind="stable")
    in_maps, cls_maps = make_in_maps(feat1, feat2, label1, order)
    nc = _get_nc()
    res = run_bass_kernel_spmd(
        nc, in_maps, core_ids=list(range(N_CORES)), trace=trace
    )
    hsegs = [res.results[i]["hseg"] for i in range(N_CORES)]
    out = finish_host(hsegs, cls_maps, feat1, feat2, label1, order)
    if trace:
        return out, res
    return out
